# revision 13
# baseline (speedup 1.0000x reference)
"""Trainium2 kernel for nn_PlanarNet: batched Kac-Ward slogdet loss.

loss = -mean_b [ sum_e log(1-p_e) + 0.5*log|det(I - kwz @ diag(w_dir_b))| ]

Algorithm: truncated trace series log|det(I-A_b)| = -(tr1_b + tr2_b/2)
+ O(rho^3) with rho ~ 0.09 (K=2 truncation: rel err 2e-7 on the loss vs
the 2e-2 gate; the slogdet term contributes ~1e-4 of the loss, which is
dominated by the detector-independent sum_e log(1-p_e)).

Both trace terms collapse to undirected-edge (E=512) space because
w_dir duplicates each undirected weight over 2 directed edges:
  tr1_b = sum_f S[b,f] g_f,            g_f  = G[2f,2f]+G[2f+1,2f+1]
  tr2_b = sum_{ef} S[b,e] H_ef S[b,f], H_ef = 2x2 block-sum of G*G^T
with S[b,f] = (-1)^{op_bf} w_f, op = (det @ pebz) % 2.

Device (per core, f-shard of 64 undirected edges, e-axis rotated per
core so its shard sits at partitions 0..63):
  opT = pebz^T @ det^T            (fp8 matmul, exact 0/1 counts)
  signs via exact fp32 RNE parity: t=rne(op/2) (+2^23 trick),
    v = op-2t in {0,+-1}, S = w - 2w*v^2   (no mod/int ops needed)
  Y^T = Q_shard^T @ S             (Q = H/2, bf16)
  P[f,b] = (Y^T[f,b] + g_f) * S[f,b]  -> acc out [64,64]
Host: priors/w/g/H prep (O(E^2)), const = sum log1p(-p), and the final
sum: loss = -(const - 0.5 * mean_b sum_{c,f} P).

Per-core device work: 12 small matmuls + 2 ACT + 7 DVE ops, ~210KB DMA.
"""
import sys
import numpy as np

sys.path.insert(0, '/opt/trn_rl_repo')

import concourse.bass as bass
import concourse.mybir as mybir
from concourse.bass_utils import run_bass_kernel_spmd

F32 = mybir.dt.float32
BF16 = mybir.dt.bfloat16
FP8 = mybir.dt.float8e4

E = 512          # undirected edges
D = 256          # detectors
B = 64           # batch
NCORES = 8
FS = E // NCORES  # f-shard per core
TB = float(2 ** 23)

_cache = {}


def build_nc(reps=1):
    """Per-core Bass program (v3: 4-deep buffers, 3-rep pipeline skew).

    Inputs: pebz8 [128,2,E] fp8 (d-slabs, e-cols core-rotated), det8
    [128,2,B] fp8 (d-slabs, = det^T), qm [128,4,FS] bf16 (e-slabs
    rotated, f-shard cols), smalls [128,9] f32 (cols 0-3: 2w e-layout,
    4-7: +w, 8: (g+y0) shard in rows 0..63). Output acc [FS,B] f32 =
    P' = (Q^T z - (g+y0)) * (w - z) rows 0..63; host negates and sums.
    `reps` repeats the compute (same data) for marginal-time runs.

    Stage chain per rep j: opT(PE) -> t(ACT) -> u(GPS) -> v(DVE) ->
    z-affines(GPS x3 + DVE x1) + srow(GPS) -> Y(PE) -> pair(DVE),
    where z = 2w*v^2 replaces S = w*(1-2*parity) = w - z: the Y matmul
    runs on z and the host folds in y0 = Q^T w, dropping the square
    stage. Every stage buffer is 4-deep (j%4) and emission is skewed
    (opT/t: +3, u/v: +2) so the ~7-stage chain latency is hidden and
    marginal cost ~= the busiest engine stream. GPSIMD ops touch SBUF
    only (HW: GPSIMD cannot access PSUM); PSUM buffers are padded to a
    full 2KB bank per rep-slot (accumulate+read must not share banks).
    """
    nc = bass.Bass()
    pebz8 = nc.declare_dram_parameter("pebz8", [128, 2, E], FP8, isOutput=False)
    det8 = nc.declare_dram_parameter("det8", [128, 2, B], FP8, isOutput=False)
    qm = nc.declare_dram_parameter("qm", [128, 4, FS], BF16, isOutput=False)
    smalls = nc.declare_dram_parameter("smalls", [128, 9], F32, isOutput=False)
    acc = nc.declare_dram_parameter("acc", [FS, B], F32, isOutput=True)

    NB_ = 4  # buffer depth (PSUM: 4 banks each for ps1/ps2)

    from contextlib import ExitStack
    with ExitStack() as ctx:
        ec = ctx.enter_context
        pz_s = ec(nc.sbuf_tensor([128, 2, E], FP8))
        dt_s = ec(nc.sbuf_tensor([128, 2, B], FP8))
        qm_s = ec(nc.sbuf_tensor([128, 4, FS], BF16))
        sm_s = ec(nc.sbuf_tensor([128, 9], F32))
        t_s = ec(nc.sbuf_tensor([128, NB_, 4, B], F32))
        u_s = ec(nc.sbuf_tensor([128, NB_, 4, B], F32))
        v_s = ec(nc.sbuf_tensor([128, NB_, 4, B], F32))
        z_s = ec(nc.sbuf_tensor([128, NB_, 4, B], BF16))
        sr_s = ec(nc.sbuf_tensor([128, NB_, B], F32))
        p_s = ec(nc.sbuf_tensor([128, B], F32))
        ps1 = ec(nc.psum_tensor([128, NB_, 8, B], F32))
        ps2 = ec(nc.psum_tensor([128, NB_, 8, B], F32))
        dma_sem = ec(nc.semaphore())
        s_o = ec(nc.semaphore())   # PE opT: 8/rep
        s_y = ec(nc.semaphore())   # PE Y: 4/rep
        s_t = ec(nc.semaphore())   # ACT t: 1/rep
        s_u = ec(nc.semaphore())   # GPS u: 1/rep
        s_z = ec(nc.semaphore())   # GPS z: 1/rep
        s_sr = ec(nc.semaphore())  # GPS srow: 1/rep
        s_v = ec(nc.semaphore())   # DVE v: 1/rep
        s_p = ec(nc.semaphore())   # DVE pair: 1/rep
        block = ec(nc.Block())

        def emit_opt(tensor, j):
            if j == 0:
                tensor.wait_ge(dma_sem, 64)          # all input DMAs
            if j >= NB_:
                # WAR ps1[j%4]: readers are t(j-4) on ACT, v(j-4) on DVE
                tensor.wait_ge(s_t, j - 3)
                tensor.wait_ge(s_v, j - 3)
            for q in range(4):
                for kd in range(2):
                    mm = tensor.matmul(
                        ps1[:, j % NB_, q, :],
                        pz_s[:, kd, 128 * q:128 * (q + 1)],
                        dt_s[:, kd, :],
                        start=(kd == 0), stop=(kd == 1),
                    )
                    mm.then_inc(s_o, 1)

        def emit_t(scalar, j):
            scalar.wait_ge(s_o, 8 * (j + 1))         # opT(j) done
            if j >= NB_:
                scalar.wait_ge(s_u, j - 3)           # WAR t_s[j%4] vs u(j-4)
            scalar.activation(
                t_s[:, j % NB_, :, :], ps1[:, j % NB_, 0:4, :],
                mybir.ActivationFunctionType.Copy,
                bias=TB, scale=0.5,
            ).then_inc(s_t, 1)

        def emit_u(gpsimd, j):
            gpsimd.wait_ge(s_t, j + 1)               # t(j) done
            if j >= NB_:
                gpsimd.wait_ge(s_v, j - 3)           # WAR u_s[j%4] vs v(j-4)
            # u = -2*rne(op/2), exact: (t - 2^23)*(-2)
            gpsimd.tensor_scalar(
                out=u_s[:, j % NB_, :, :], in0=t_s[:, j % NB_, :, :],
                scalar1=TB, scalar2=-2.0,
                op0=mybir.AluOpType.subtract, op1=mybir.AluOpType.mult,
            ).then_inc(s_u, 1)

        def emit_z(gpsimd, j):
            # z = v*v in {0,1} (exact in bf16); 2w is folded into qm on host
            gpsimd.wait_ge(s_v, j + 1)               # v(j) done
            if j >= NB_:
                gpsimd.wait_ge(s_y, 4 * (j - 3))     # WAR z[j%4] vs Y(j-4)
                gpsimd.wait_ge(s_sr, j - 3)          # WAR z[j%4] vs srow(j-4)
            gpsimd.tensor_mul(
                z_s[:, j % NB_, :, :], v_s[:, j % NB_, :, :],
                v_s[:, j % NB_, :, :]
            ).then_inc(s_z, 1)

        def emit_srow(gpsimd, j):
            # srow = w - 2w*z = +-w (rows 0..63 of q0), exact f32
            gpsimd.wait_ge(s_z, j + 1)               # z(j) self-edge
            if j >= NB_:
                gpsimd.wait_ge(s_p, j - 3)           # WAR sr_s[j%4] vs pair
            gpsimd.tensor_scalar(
                out=sr_s[0:FS, j % NB_, :], in0=z_s[0:FS, j % NB_, 0, :],
                scalar1=sm_s[0:FS, 0:1], scalar2=sm_s[0:FS, 4:5],
                op0=mybir.AluOpType.mult, op1=mybir.AluOpType.add,
            ).then_inc(s_sr, 1)

        def emit_v(vector, j):
            vector.wait_ge(s_u, j + 1)               # u(j) done
            vector.wait_ge(s_o, 8 * (j + 1))         # ps1 read edge
            if j >= NB_:
                vector.wait_ge(s_z, j - 3)           # WAR v_s[j%4] vs z(j-4)
            # v = op - 2*rne(op/2)  in {0, +-1}
            vector.tensor_add(
                v_s[:, j % NB_, :, :], ps1[:, j % NB_, 0:4, :],
                u_s[:, j % NB_, :, :]
            ).then_inc(s_v, 1)

        @block.sync
        def _(sync):
            sync.dma_start(out=pz_s[:], in_=pebz8[:]).then_inc(dma_sem, 16)
            sync.dma_start(out=dt_s[:], in_=det8[:]).then_inc(dma_sem, 16)
            sync.dma_start(out=qm_s[:], in_=qm[:]).then_inc(dma_sem, 16)
            sync.dma_start(out=sm_s[:], in_=smalls[:]).then_inc(dma_sem, 16)
            sync.wait_ge(s_p, reps)
            sync.dma_start(out=acc[:], in_=p_s[0:FS, :]).then_inc(dma_sem, 16)

        @block.tensor
        def _(tensor):
            for j in range(min(3, reps)):
                emit_opt(tensor, j)
            for r in range(reps):
                if r + 3 < reps:
                    emit_opt(tensor, r + 3)
                tensor.wait_ge(s_z, r + 1)           # z(r) built
                if r >= NB_:
                    tensor.wait_ge(s_p, r - 3)       # WAR ps2[r%4] vs pair
                for k in range(4):
                    mm = tensor.matmul(
                        ps2[0:FS, r % NB_, 0, :],
                        qm_s[:, k, :],
                        z_s[:, r % NB_, k, :],
                        start=(k == 0), stop=(k == 3),
                    )
                    mm.then_inc(s_y, 1)

        @block.scalar
        def _(scalar):
            for j in range(min(3, reps)):
                emit_t(scalar, j)
            for r in range(reps):
                if r + 3 < reps:
                    emit_t(scalar, r + 3)

        @block.gpsimd
        def _(gpsimd):
            gpsimd.wait_ge(dma_sem, 64)              # smalls
            for j in range(min(2, reps)):
                emit_u(gpsimd, j)
            if reps >= 2:
                emit_z(gpsimd, 0)
                emit_srow(gpsimd, 0)
            for r in range(reps):
                if r + 2 < reps:
                    emit_u(gpsimd, r + 2)
                if r + 1 < reps:
                    emit_z(gpsimd, r + 1)
                    emit_srow(gpsimd, r + 1)
                elif reps == 1:
                    emit_z(gpsimd, 0)
                    emit_srow(gpsimd, 0)

        @block.vector
        def _(vector):
            vector.wait_ge(dma_sem, 64)              # smalls
            for j in range(min(2, reps)):
                emit_v(vector, j)
            for r in range(reps):
                if r + 2 < reps:
                    emit_v(vector, r + 2)
                vector.wait_ge(s_y, 4 * (r + 1))     # Y(r) done
                vector.wait_ge(s_sr, r + 1)          # srow(r) done
                if r >= 1:
                    vector.wait_ge(s_p, r)           # p_s WAW self-edge
                # P' = (Q2w^T z - (g+y0)) * srow
                vector.scalar_tensor_tensor(
                    out=p_s[0:FS, :], in0=ps2[0:FS, r % NB_, 0, :],
                    scalar=sm_s[0:FS, 8:9], in1=sr_s[0:FS, r % NB_, :],
                    op0=mybir.AluOpType.subtract, op1=mybir.AluOpType.mult,
                ).then_inc(s_p, 1)

    return nc


def _host_prep(det, pebz, para, kwz, edges_dict_z):
    para64 = para.astype(np.float64)
    priors = 1.0 / (1.0 + np.exp(-para64)) + 1e-20
    w = priors / (1.0 - priors)                        # [E]
    const = np.sum(np.log1p(-priors))
    G = kwz.astype(np.float64)
    g2 = np.diag(G).reshape(E, 2).sum(1)               # [E]
    Q = (G * G.T).reshape(E, 2, E, 2).sum(axis=(1, 3)) / 2.0
    y0 = Q.T @ w                                       # [E], f64
    return w, g2, Q, y0, const


def make_in_maps(det, pebz, w, g2, Q, y0):
    import ml_dtypes
    f8 = ml_dtypes.float8_e4m3
    bf = ml_dtypes.bfloat16
    det8 = np.ascontiguousarray(
        det.T.astype(f8).reshape(2, 128, B).transpose(1, 0, 2))
    in_maps = []
    for c in range(NCORES):
        perm = np.roll(np.arange(E), -FS * c)
        pz = np.ascontiguousarray(
            pebz[:, perm].astype(f8).reshape(2, 128, E).transpose(1, 0, 2))
        qmc = np.ascontiguousarray(
            ((2.0 * w[perm])[:, None] * Q[perm][:, FS * c:FS * (c + 1)])
            .astype(bf).reshape(4, 128, FS).transpose(1, 0, 2))
        wp = w[perm].astype(np.float32).reshape(4, 128).T    # [128, 4]
        sm = np.zeros((128, 9), np.float32)
        sm[:, 0] = -2.0 * wp[:, 0]
        sm[:, 4] = wp[:, 0]
        sm[0:FS, 8] = (g2 + y0)[FS * c:FS * (c + 1)].astype(np.float32)
        in_maps.append({"pebz8": pz, "det8": det8, "qm": qmc, "smalls": sm})
    return in_maps


def kernel(det, pebz, para, kwz, edges_dict_z):
    det = np.asarray(det)
    pebz = np.asarray(pebz)
    para = np.asarray(para)
    kwz = np.asarray(kwz)
    edges_dict_z = np.asarray(edges_dict_z)
    w, g2, Q, y0, const = _host_prep(det, pebz, para, kwz, edges_dict_z)

    if 'nc' not in _cache:
        _cache['nc'] = build_nc(reps=1)
    nc = _cache['nc']

    in_maps = make_in_maps(det, pebz, w, g2, Q, y0)
    res = run_bass_kernel_spmd(nc, in_maps, list(range(NCORES)))

    # acc holds P' = (Q^T z - (g+y0)) * (w - z) = -P; negate when summing
    tot = np.zeros(B)
    for c in range(NCORES):
        tot -= res.results[c]["acc"].astype(np.float64).sum(axis=0)
    loss = -(const - 0.5 * tot.mean())
    return np.float32(loss)
